# revision 24
# baseline (speedup 1.0000x reference)
"""2-layer GATv2 + global mean pool + linear head, on 8 Trainium2 NeuronCores.

Strategy (dst-sharded, degree-balanced static schedule):
  - Nodes are relabeled by a degree-balanced bin-packing into groups of <=128
    destination nodes such that every group has nearly equal incident-edge
    count.  This makes the per-group chunk count S uniform, so one SPMD
    program works for all 8 cores.
  - Core c owns nodes [c*own, (c+1)*own) (new ids).  Edges (with self loops)
    are assigned by destination owner, laid out as groups x S chunks of 128
    edge slots.
  - Per chunk: indirect-DMA gather of xl[src] and xr[dst] rows (bf16),
    z = G + V, leaky-relu, per-head score = reduce(l * att), p = exp(score),
    indicator = is_equal(dstloc, iota) and one PE matmul accumulates both
    the weighted feature sums and the softmax denominators into PSUM.
  - Per group: divide by denominators, add bias, ELU -> h row block.
  - Layer tables xl = x@W_l + b_l are built on device; the layer-2 source
    table is AllGather'ed across cores.  Final pooling partial sums are
    AllReduce'd and every core computes the tiny output head.
"""

import sys

for _p in ("/opt/trn_rl_repo",):
    if _p not in sys.path:
        sys.path.insert(0, _p)

import numpy as np
import ml_dtypes

BF = ml_dtypes.bfloat16

import concourse.bass as bass
import concourse.mybir as mybir
from concourse.tile import TileContext
from concourse.bass_utils import run_bass_kernel_spmd
from concourse.masks import make_identity

F32 = mybir.dt.float32
BF16 = mybir.dt.bfloat16
I32 = mybir.dt.int32
P = 128
NCORES = 8
NUM_GRAPHS = 64
NEG_SLOPE = 0.2


# ---------------------------------------------------------------- prof hook
def _install_profhook():
    """Provide antenv.axon_hooks (absent in this image) so trace=True works."""
    import types

    if "antenv.axon_hooks" in sys.modules:
        return
    try:
        from trn_agent_boot.trn_boot import _ntff_profile_via_ctypes
    except Exception:
        return
    mod = types.ModuleType("antenv.axon_hooks")
    mod._hook = None
    mod.set_axon_ntff_profile_hook = lambda h: setattr(mod, "_hook", h)
    mod.get_axon_ntff_profile_hook = lambda: mod._hook
    sys.modules["antenv.axon_hooks"] = mod
    try:
        mod._hook = _ntff_profile_via_ctypes("/opt/axon/libaxon_pjrt.so")
    except Exception:
        mod._hook = None


# ---------------------------------------------------------------- wait split
def _split_waits(nc, max_waits=1):
    """walrus TPB_CTRL codegen rejects >1 sync-wait per instruction; move
    extras onto preceding NoOps on the same engine."""
    n_added = 0
    for fn in nc.m.functions:
        for blk in fn.blocks:
            new_insts = []
            for inst in blk.instructions:
                si = getattr(inst, "sync_info", None)
                waits = list(si.on_wait) if si is not None and si.on_wait else []
                if len(waits) > max_waits:
                    extra = waits[:-max_waits]
                    for i in range(0, len(extra), max_waits):
                        chunk = extra[i : i + max_waits]
                        nop = mybir.InstNoOp(
                            name=f"{inst.name}_wsplit{n_added}",
                            engine=inst.engine,
                            ins=[],
                            outs=[],
                            sync_info=mybir.SyncInfo(on_wait=chunk, on_update=[]),
                        )
                        n_added += 1
                        new_insts.append(nop)
                    si.on_wait = waits[-max_waits:]
                new_insts.append(inst)
            blk.instructions = new_insts
    return n_added


# ---------------------------------------------------------------- host prep
def _prep(x, edge_index, batch, ncores):
    """Degree-balanced relabeling + per-core static edge layout."""
    N = x.shape[0]
    own = N // ncores
    gfull, rem = divmod(own, P)
    ngroups = gfull + (1 if rem else 0)

    src = np.concatenate([edge_index[0].astype(np.int64), np.arange(N)])
    dst = np.concatenate([edge_index[1].astype(np.int64), np.arange(N)])
    deg = np.bincount(dst, minlength=N)

    # bins: per core, gfull bins of cap P then (if rem) one bin of cap rem
    caps = []
    for c in range(ncores):
        caps += [P] * gfull + ([rem] if rem else [])
    nbins = len(caps)

    import heapq

    heap = [(0, b) for b in range(nbins)]
    heapq.heapify(heap)
    bin_nodes = [[] for _ in range(nbins)]
    order = np.argsort(-deg, kind="stable")
    for node in order:
        while True:
            s, b = heapq.heappop(heap)
            if len(bin_nodes[b]) < caps[b]:
                break
        bin_nodes[b].append(node)
        if len(bin_nodes[b]) < caps[b]:
            heapq.heappush(heap, (s + int(deg[node]), b))

    perm = np.empty(N, np.int64)  # perm[new] = old
    for b in range(nbins):
        c, g = divmod(b, ngroups)
        base = c * own + g * P
        nodes = bin_nodes[b]
        perm[base : base + len(nodes)] = nodes
    inv = np.empty(N, np.int64)
    inv[perm] = np.arange(N)

    bin_sums = np.array([deg[bin_nodes[b]].sum() for b in range(nbins)])
    S = int(np.ceil(bin_sums.max() / P))
    nchunk = ngroups * S

    new_src = inv[src]
    new_dst = inv[dst]
    core_of = new_dst // own

    per_core = []
    for c in range(ncores):
        m = core_of == c
        es = new_src[m]
        ed = new_dst[m] - c * own  # own-local
        eg = np.minimum(ed // P, ngroups - 1)
        eslot = ed - eg * P
        o = np.lexsort((eslot, eg))
        es, ed, eg, eslot = es[o], ed[o], eg[o], eslot[o]

        counts = np.bincount(eg, minlength=ngroups)
        assert counts.max() <= S * P, (counts.max(), S * P)
        gstart = np.zeros(ngroups, np.int64)
        gstart[1:] = np.cumsum(counts)[:-1]
        jw = np.arange(len(es)) - gstart[eg]
        fpos = eg * S * P + jw

        flat_src = np.zeros(nchunk * P, np.int32)
        flat_dst = np.zeros(nchunk * P, np.int32)
        flat_loc = np.full(nchunk * P, 255.0, np.float32)
        flat_src[fpos] = es
        flat_dst[fpos] = ed
        flat_loc[fpos] = eslot

        srcidx = flat_src.reshape(nchunk, P).T.copy()
        dstidx = flat_dst.reshape(nchunk, P).T.copy()
        dstloc = flat_loc.reshape(nchunk, P).T.astype(BF)

        xT_own = np.ascontiguousarray(x[perm[c * own : (c + 1) * own]].T).astype(BF)

        bl = np.full((P, ngroups), 255.0, np.float32)
        for g in range(ngroups):
            size = P if (g < gfull or rem == 0) else rem
            ids = perm[c * own + g * P : c * own + g * P + size]
            bl[:size, g] = batch[ids]
        batchloc = bl.astype(BF)

        per_core.append(
            dict(
                srcidx=srcidx,
                dstidx=dstidx,
                dstloc=dstloc,
                xT_own=xT_own,
                batchloc=batchloc,
            )
        )

    meta = dict(
        N=N,
        own=own,
        ngroups=ngroups,
        gfull=gfull,
        rem=rem,
        S=S,
        nchunk=nchunk,
        ncores=ncores,
    )
    return per_core, meta, perm


# ---------------------------------------------------------------- kernel build
def _gsize(meta, g):
    return P if (g < meta["gfull"] or meta["rem"] == 0) else meta["rem"]


def _build(meta, heads1=8, heads2=1, debug=False):
    N = meta["N"]
    own = meta["own"]
    ngroups = meta["ngroups"]
    S = meta["S"]
    nchunk = meta["nchunk"]
    ncores = meta["ncores"]
    D = 128

    nc = bass.Bass(target_bir_lowering=False, debug=True)

    # ---- external inputs (per core)
    xT_in = nc.declare_dram_parameter("xT_own", [P, own], BF16, isOutput=False)
    srcidx_in = nc.declare_dram_parameter("srcidx", [P, nchunk], I32, isOutput=False)
    dstidx_in = nc.declare_dram_parameter("dstidx", [P, nchunk], I32, isOutput=False)
    dstloc_in = nc.declare_dram_parameter("dstloc", [P, nchunk], BF16, isOutput=False)
    batchloc_in = nc.declare_dram_parameter(
        "batchloc", [P, ngroups], BF16, isOutput=False
    )
    # consts (replicated)
    wnames = [
        ("W1_l", [P, D]), ("W1_r", [P, D]), ("W2_l", [P, D]), ("W2_r", [P, D]),
        ("att1_rep", [P, D]), ("att2_rep", [P, D]),
        ("bias1_rep", [P, D]), ("bias2_rep", [P, D]),
        ("b1_l", [1, D]), ("b1_r", [1, D]), ("b2_l", [1, D]), ("b2_r", [1, D]),
        ("iota128", [P, P]), ("iota64", [P, NUM_GRAPHS]),
        ("W3", [P, 10]), ("b3row", [1, 10]), ("ones1", [1, P]),
    ]
    w_in = {n: nc.declare_dram_parameter(n, sh, BF16, isOutput=False) for n, sh in wnames}
    out_t = nc.declare_dram_parameter("out", [NUM_GRAPHS, 10], F32, isOutput=True)
    dbg = {}
    if debug:
        for n, sh in [("dbg_xl1", [own, D]), ("dbg_xr1", [own, D]),
                      ("dbg_xl1full", [own * ncores, D]), ("dbg_h1T", [P, own]),
                      ("dbg_xl2", [own, D]), ("dbg_pool", [NUM_GRAPHS, 129])]:
            dt = F32 if n == "dbg_pool" else BF16
            dbg[n] = nc.declare_dram_parameter(n, sh, dt, isOutput=True)
        dbg["dbg_red"] = nc.declare_dram_parameter(
            "dbg_red", [NUM_GRAPHS, 129], F32, isOutput=True)

    # ---- internal DRAM
    xl1_own = nc.dram_tensor("xl1_own", [own, D], BF16)
    xr1_own = nc.dram_tensor("xr1_own", [own, D], BF16)
    xl1_full = nc.dram_tensor("xl1_full", [own * ncores, D], BF16, addr_space="Shared")
    xl2_own = nc.dram_tensor("xl2_own", [own, D], BF16)
    xr2_own = nc.dram_tensor("xr2_own", [own, D], BF16)
    xl2_full = nc.dram_tensor("xl2_full", [own * ncores, D], BF16, addr_space="Shared")
    pool_stage = nc.dram_tensor("pool_stage", [NUM_GRAPHS, 129], F32)
    pool_red = nc.dram_tensor("pool_red", [NUM_GRAPHS, 129], F32, addr_space="Shared")

    # ---- persistent SBUF
    sb = {}
    def persist(name, shape, dtype):
        sb[name] = nc.alloc_sbuf_tensor(name, shape, dtype)
        return sb[name]

    xT_sb = persist("xT_sb", [P, own], BF16)
    srcidx_sb = persist("srcidx_sb", [P, nchunk], I32)
    dstidx_sb = persist("dstidx_sb", [P, nchunk], I32)
    dstloc_sb = persist("dstloc_sb", [P, nchunk], BF16)
    batchloc_sb = persist("batchloc_sb", [P, ngroups], BF16)
    h1T_sb = persist("h1T_sb", [P, own], BF16)
    ident_sb = persist("ident_sb", [P, P], BF16)
    w_sb = {n: persist(n + "_sb", sh, BF16) for n, sh in wnames}

    def collective(kind, op, ins, outs):
        nc.gpsimd.collective_compute(
            kind, op, replica_groups=[list(range(ncores))], ins=ins, outs=outs
        )

    # ================= TC-load: all constant loads =================
    with TileContext(nc) as tc:
        nc.sync.dma_start(out=xT_sb[:], in_=xT_in[:])
        nc.sync.dma_start(out=srcidx_sb[:], in_=srcidx_in[:])
        nc.sync.dma_start(out=dstidx_sb[:], in_=dstidx_in[:])
        nc.sync.dma_start(out=dstloc_sb[:], in_=dstloc_in[:])
        nc.sync.dma_start(out=batchloc_sb[:], in_=batchloc_in[:])
        for n, _sh in wnames:
            nc.sync.dma_start(out=w_sb[n][:], in_=w_in[n][:])
        with tc.tile_pool(name="idp", bufs=1) as idp:
            idt = idp.tile([P, P], F32)
            make_identity(nc, idt[:])
            nc.vector.tensor_copy(out=ident_sb[:], in_=idt[:])

    # ================= TC0: build xl1/xr1 tables =================
    def build_tables(tc, srcT_sb, Wl, Wr, bl, br, out_l, out_r):
        with (
            tc.tile_pool(name="tp", bufs=3) as tp,
            tc.tile_pool(name="tpp", bufs=3, space="PSUM") as tpp,
        ):
            for g in range(ngroups):
                w = _gsize(meta, g)
                for W, brow, dest in ((Wl, bl, out_l), (Wr, br, out_r)):
                    ps = tpp.tile([P, D], F32, tag="ps")
                    nc.tensor.matmul(
                        out=ps[:w, :],
                        lhsT=srcT_sb[:, g * P : g * P + w],
                        rhs=w_sb[W][:],
                        start=True,
                        stop=False,
                    )
                    nc.tensor.matmul(
                        out=ps[:w, :],
                        lhsT=w_sb["ones1"][:, :w],
                        rhs=w_sb[brow][:],
                        start=False,
                        stop=True,
                    )
                    ot = tp.tile([P, D], BF16, tag="ot")
                    nc.scalar.activation(
                        out=ot[:w, :], in_=ps[:w, :],
                        func=mybir.ActivationFunctionType.Copy,
                    )
                    nc.sync.dma_start(
                        out=dest[g * P : g * P + w, :], in_=ot[:w, :]
                    )

    with TileContext(nc) as tc:
        build_tables(tc, xT_sb, "W1_l", "W1_r", "b1_l", "b1_r", xl1_own, xr1_own)

    # ================= edge layer =================
    def edge_layer(tc, xl_full_t, xr_own_t, att_rep, bias_rep, heads, pool_ctx=None):
        """one GATv2 layer over the static edge schedule.
        pool_ctx: None for layer1 (writes h1T_sb); else (pool_psum,) for layer2."""
        C = D // heads
        NH = heads
        with (
            tc.tile_pool(name="gv", bufs=6) as gv,
            tc.tile_pool(name="work", bufs=3) as work,
            tc.tile_pool(name="rhsp", bufs=3) as rhsp,
            tc.tile_pool(name="sc", bufs=3) as scp,
            tc.tile_pool(name="ep", bufs=2) as ep,
            tc.tile_pool(name="aggp", bufs=2, space="PSUM") as aggp,
            tc.tile_pool(name="tpsum", bufs=2, space="PSUM") as tpsum,
        ):
            for g in range(ngroups):
                w = _gsize(meta, g)
                agg = aggp.tile([P, D + NH], F32, tag="agg")
                for j in range(S):
                    k = g * S + j
                    G = gv.tile([P, D], BF16, tag="G")
                    V = gv.tile([P, D], BF16, tag="V")
                    nc.gpsimd.indirect_dma_start(
                        out=G[:], out_offset=None, in_=xl_full_t[:],
                        in_offset=bass.IndirectOffsetOnAxis(
                            ap=srcidx_sb[:, k : k + 1], axis=0),
                    )
                    nc.gpsimd.indirect_dma_start(
                        out=V[:], out_offset=None, in_=xr_own_t[:],
                        in_offset=bass.IndirectOffsetOnAxis(
                            ap=dstidx_sb[:, k : k + 1], axis=0),
                    )
                    z = work.tile([P, D], BF16, tag="z")
                    nc.vector.tensor_tensor(out=z[:], in0=G[:], in1=V[:],
                                            op=mybir.AluOpType.add)
                    zs = work.tile([P, D], BF16, tag="zs")
                    nc.vector.tensor_scalar(out=zs[:], in0=z[:],
                                            scalar1=NEG_SLOPE, scalar2=None,
                                            op0=mybir.AluOpType.mult)
                    lr = work.tile([P, D], BF16, tag="lr")
                    nc.vector.tensor_tensor(out=lr[:], in0=z[:], in1=zs[:],
                                            op=mybir.AluOpType.max)
                    m = work.tile([P, D], BF16, tag="m")
                    nc.vector.tensor_tensor(out=m[:], in0=lr[:],
                                            in1=w_sb[att_rep][:],
                                            op=mybir.AluOpType.mult)
                    score = scp.tile([P, NH], F32, tag="score")
                    nc.vector.tensor_reduce(
                        out=score[:],
                        in_=m[:].rearrange("p (h c) -> p h c", h=NH),
                        axis=mybir.AxisListType.X, op=mybir.AluOpType.add,
                    )
                    rhs = rhsp.tile([P, D + NH], BF16, tag="rhs")
                    nc.scalar.activation(
                        out=rhs[:, D : D + NH], in_=score[:],
                        func=mybir.ActivationFunctionType.Exp,
                    )
                    p_b = rhs[:, D : D + NH].unsqueeze(2).broadcast_to([P, NH, C])
                    nc.vector.tensor_tensor(
                        out=rhs[:, :D].rearrange("p (h c) -> p h c", h=NH),
                        in0=G[:].rearrange("p (h c) -> p h c", h=NH),
                        in1=p_b, op=mybir.AluOpType.mult)
                    ind = work.tile([P, P], BF16, tag="ind")
                    nc.vector.tensor_tensor(
                        out=ind[:],
                        in0=dstloc_sb[:, k : k + 1].to_broadcast([P, P]),
                        in1=w_sb["iota128"][:],
                        op=mybir.AluOpType.is_equal,
                    )
                    nc.tensor.matmul(out=agg[:], lhsT=ind[:], rhs=rhs[:],
                                     start=(j == 0), stop=(j == S - 1))
                # ---- group epilogue
                den = ep.tile([P, NH], F32, tag="den")
                nc.vector.tensor_scalar(out=den[:], in0=agg[:, D : D + NH],
                                        scalar1=1e-30, scalar2=None,
                                        op0=mybir.AluOpType.max)
                rec = ep.tile([P, NH], F32, tag="rec")
                nc.vector.reciprocal(out=rec[:], in_=den[:])
                rec_b = rec[:].unsqueeze(2).broadcast_to([P, NH, C])
                outn = ep.tile([P, D], F32, tag="outn")
                nc.vector.tensor_tensor(
                    out=outn[:].rearrange("p (h c) -> p h c", h=NH),
                    in0=agg[:, :D].rearrange("p (h c) -> p h c", h=NH),
                    in1=rec_b, op=mybir.AluOpType.mult)
                nc.vector.tensor_tensor(out=outn[:], in0=outn[:],
                                        in1=w_sb[bias_rep][:],
                                        op=mybir.AluOpType.add)
                # elu: pos = max(outn,0); neg = min(outn,0); h = pos + (exp(neg)-1)
                neg = ep.tile([P, D], F32, tag="neg")
                nc.vector.tensor_scalar(out=neg[:], in0=outn[:], scalar1=0.0,
                                        scalar2=None, op0=mybir.AluOpType.min)
                en = ep.tile([P, D], F32, tag="en")
                nc.scalar.activation(out=en[:], in_=neg[:],
                                     func=mybir.ActivationFunctionType.Exp)
                nc.vector.tensor_scalar(out=en[:], in0=en[:], scalar1=-1.0,
                                        scalar2=None, op0=mybir.AluOpType.add)
                nc.vector.tensor_scalar(out=outn[:], in0=outn[:], scalar1=0.0,
                                        scalar2=None, op0=mybir.AluOpType.max)
                h = ep.tile([P, D], BF16, tag="h")
                nc.vector.tensor_tensor(out=h[:], in0=outn[:], in1=en[:],
                                        op=mybir.AluOpType.add)
                if pool_ctx is None:
                    # h1T resident: transpose h -> [feat, nodes]
                    tps = tpsum.tile([P, P], BF16, tag="tps")
                    nc.tensor.transpose(out=tps[:, :w], in_=h[:w, :],
                                        identity=ident_sb[:w, :w])
                    nc.scalar.activation(
                        out=h1T_sb[:, g * P : g * P + w], in_=tps[:, :w],
                        func=mybir.ActivationFunctionType.Copy,
                    )
                else:
                    (pool_psum,) = pool_ctx
                    pind = work.tile([P, NUM_GRAPHS], BF16, tag="pind")
                    nc.vector.tensor_tensor(
                        out=pind[:],
                        in0=batchloc_sb[:, g : g + 1].to_broadcast([P, NUM_GRAPHS]),
                        in1=w_sb["iota64"][:],
                        op=mybir.AluOpType.is_equal,
                    )
                    prhs = rhsp.tile([P, D + 1], BF16, tag="prhs")
                    nc.vector.tensor_copy(out=prhs[:, :D], in_=h[:])
                    nc.gpsimd.memset(prhs[:, D : D + 1], 1.0)
                    nc.tensor.matmul(out=pool_psum[:], lhsT=pind[:], rhs=prhs[:],
                                     start=(g == 0), stop=(g == ngroups - 1))

    with TileContext(nc) as tc:
        collective("AllGather", mybir.AluOpType.bypass, [xl1_own[:]], [xl1_full[:]])
        edge_layer(tc, xl1_full, xr1_own, "att1_rep", "bias1_rep", 8)

    with TileContext(nc) as tc:
        build_tables(tc, h1T_sb, "W2_l", "W2_r", "b2_l", "b2_r", xl2_own, xr2_own)

    with TileContext(nc) as tc:
        collective("AllGather", mybir.AluOpType.bypass, [xl2_own[:]], [xl2_full[:]])
        with tc.tile_pool(name="poolp", bufs=1, space="PSUM") as poolp, \
             tc.tile_pool(name="pstg", bufs=1) as pstg:
            pool_psum = poolp.tile([NUM_GRAPHS, 129], F32)
            edge_layer(tc, xl2_full, xr2_own, "att2_rep", "bias2_rep", 1,
                       pool_ctx=(pool_psum,))
            stg = pstg.tile([NUM_GRAPHS, 129], F32)
            nc.scalar.activation(out=stg[:], in_=pool_psum[:],
                                 func=mybir.ActivationFunctionType.Copy)
            nc.sync.dma_start(out=pool_stage[:], in_=stg[:])

    # ================= final head =================
    with TileContext(nc) as tc:
        collective("AllReduce", mybir.AluOpType.add, [pool_stage[:]], [pool_red[:]])
        with (
            tc.tile_pool(name="fin", bufs=1) as fin,
            tc.tile_pool(name="finp", bufs=1, space="PSUM") as finp,
        ):
            red = fin.tile([NUM_GRAPHS, 129], F32)
            nc.sync.dma_start(out=red[:], in_=pool_red[:])
            if debug:
                nc.sync.dma_start(out=dbg["dbg_red"][:], in_=red[:])
            cnt = fin.tile([NUM_GRAPHS, 1], F32)
            nc.vector.tensor_scalar(out=cnt[:], in0=red[:, 128:129], scalar1=1.0,
                                    scalar2=None, op0=mybir.AluOpType.max)
            rc = fin.tile([NUM_GRAPHS, 1], F32)
            nc.vector.reciprocal(out=rc[:], in_=cnt[:])
            pooled = fin.tile([NUM_GRAPHS, D], BF16)
            nc.vector.tensor_tensor(out=pooled[:], in0=red[:, :D],
                                    in1=rc[:].to_broadcast([NUM_GRAPHS, D]),
                                    op=mybir.AluOpType.mult)
            tp = finp.tile([P, NUM_GRAPHS], BF16)
            nc.tensor.transpose(out=tp[:], in_=pooled[:],
                                identity=ident_sb[:NUM_GRAPHS, :NUM_GRAPHS])
            pooledT = fin.tile([P, NUM_GRAPHS], BF16)
            nc.scalar.activation(out=pooledT[:], in_=tp[:],
                                 func=mybir.ActivationFunctionType.Copy)
            ops = finp.tile([NUM_GRAPHS, 10], F32)
            nc.tensor.matmul(out=ops[:], lhsT=pooledT[:], rhs=w_sb["W3"][:],
                             start=True, stop=False)
            nc.tensor.matmul(out=ops[:], lhsT=w_sb["ones1"][:, :NUM_GRAPHS],
                             rhs=w_sb["b3row"][:], start=False, stop=True)
            fout = fin.tile([NUM_GRAPHS, 10], F32)
            nc.scalar.activation(out=fout[:], in_=ops[:],
                                 func=mybir.ActivationFunctionType.Copy)
            nc.sync.dma_start(out=out_t[:], in_=fout[:])
        if debug:
            nc.sync.dma_start(out=dbg["dbg_xl1"][:], in_=xl1_own[:])
            nc.sync.dma_start(out=dbg["dbg_xr1"][:], in_=xr1_own[:])
            nc.sync.dma_start(out=dbg["dbg_xl1full"][:], in_=xl1_full[:])
            nc.sync.dma_start(out=dbg["dbg_h1T"][:], in_=h1T_sb[:])
            nc.sync.dma_start(out=dbg["dbg_xl2"][:], in_=xl2_own[:])
            nc.sync.dma_start(out=dbg["dbg_pool"][:], in_=pool_stage[:])

    _split_waits(nc)
    return nc


# ---------------------------------------------------------------- entry point
def _run(x, edge_index, batch, W1_l, b1_l, W1_r, b1_r, att1, bias1,
         W2_l, b2_l, W2_r, b2_r, att2, bias2, W3, b3, ncores=NCORES,
         debug=False, trace=False):
    x = np.asarray(x, np.float32)
    per_core, meta, perm = _prep(np.asarray(x), np.asarray(edge_index),
                                 np.asarray(batch), ncores)
    D = 128
    consts = dict(
        W1_l=np.asarray(W1_l, np.float32).astype(BF),
        W1_r=np.asarray(W1_r, np.float32).astype(BF),
        W2_l=np.asarray(W2_l, np.float32).astype(BF),
        W2_r=np.asarray(W2_r, np.float32).astype(BF),
        att1_rep=np.tile(np.asarray(att1, np.float32).reshape(1, D), (P, 1)).astype(BF),
        att2_rep=np.tile(np.asarray(att2, np.float32).reshape(1, D), (P, 1)).astype(BF),
        bias1_rep=np.tile(np.asarray(bias1, np.float32).reshape(1, D), (P, 1)).astype(BF),
        bias2_rep=np.tile(np.asarray(bias2, np.float32).reshape(1, D), (P, 1)).astype(BF),
        b1_l=np.asarray(b1_l, np.float32).reshape(1, D).astype(BF),
        b1_r=np.asarray(b1_r, np.float32).reshape(1, D).astype(BF),
        b2_l=np.asarray(b2_l, np.float32).reshape(1, D).astype(BF),
        b2_r=np.asarray(b2_r, np.float32).reshape(1, D).astype(BF),
        iota128=np.tile(np.arange(P, dtype=np.float32).reshape(1, P), (P, 1)).astype(BF),
        iota64=np.tile(np.arange(NUM_GRAPHS, dtype=np.float32).reshape(1, NUM_GRAPHS), (P, 1)).astype(BF),
        W3=np.asarray(W3, np.float32).astype(BF),
        b3row=np.asarray(b3, np.float32).reshape(1, 10).astype(BF),
        ones1=np.ones((1, P), np.float32).astype(BF),
    )
    nc = _build(meta, debug=debug)
    in_maps = []
    for c in range(ncores):
        m = dict(per_core[c])
        m.update(consts)
        in_maps.append(m)
    if trace:
        _install_profhook()
    res = run_bass_kernel_spmd(nc, in_maps, core_ids=list(range(ncores)),
                               trace=trace)
    return res.results[0]["out"].astype(np.float32), (res, per_core, meta, perm)


def kernel(**inputs):
    out, _res = _run(**inputs)
    return out
